# revision 1
# baseline (speedup 1.0000x reference)
"""ColorRandomizer Trainium2 kernel: brightness/contrast/saturation/hue on 8 cores.

Data-parallel: 4 images per core. Per image:
  ph1  x1 = min(x*bf, 1)                (DVE tensor_scalar, fp32->fp16, + free accum for mean)
  ph2  x2 = clip01(cf*x1 + (1-cf)*mean) (ACT relu affine + DVE min)
  ph3  x3 = clip01(sf*x2 + (1-sf)*gray(x2))
  ph4  HSV hue shift, reformulated:
         J = cr*H6 built w/o division via case-select (copy_predicated),
         i6 = J * exp(-ln(cr+eps)) + 6*hf,
         out_c = min(minc + cr*relu(min(|i6+a_c|,|i6+a_c-6|) - 1), maxc)
       (tent identity for HSV->RGB; no floor/mod needed for |hf|<=1/3)
Storage fp16 (validated absmax ~6.5e-3 vs fp32 reference), scalars/accums fp32.
"""
import sys

for _p in ("/opt/trn_rl_repo",):
    if _p not in sys.path:
        sys.path.append(_p)

import numpy as np
from concourse import bass, bacc, mybir, tile, bass_isa
from concourse.bass_utils import run_bass_kernel_spmd

F32 = mybir.dt.float32
F16 = mybir.dt.float16
OP = mybir.AluOpType
AF = mybir.ActivationFunctionType

NIMG = 4          # images per core
H, W = 480, 640
NPIX = H * W      # 307200
F = NPIX // 128   # 2400 free elems per partition per plane
F3 = 3 * F
GRAY_W = (0.299, 0.587, 0.114)

TRACE = False     # test.py flips this for profiling
_CACHE = {}


def _build():
    nc = bacc.Bacc(None, target_bir_lowering=False)
    x_h = nc.declare_dram_parameter("x", [NIMG, 3, H, W], F32, isOutput=False)
    fac_h = nc.declare_dram_parameter("fac", [NIMG, 8], F32, isOutput=False)
    y_h = nc.declare_dram_parameter("y", [NIMG, 3, H, W], F32, isOutput=True)

    dma = nc.sync  # HWDGE

    # activation float biases must exist as const APs
    for v in (1e-30, -1.0, 3.0, -3.0, -5.0, -7.0):
        t = nc.alloc_sbuf_tensor(f"cst-{v}", [128, 1], F32)
        nc.gpsimd.memset(t.ap(), v)
        nc.const_aps.aps[(F32, v)] = t.ap()
    nc.all_engine_barrier()

    with tile.TileContext(nc) as tc:
        with tc.tile_pool(name="p", bufs=1) as pool:
            # broadcast per-image factors to all partitions once
            fac1 = pool.tile([1, NIMG * 8], F32)
            dma.dma_start(fac1[:], fac_h[:].flatten()[None, :])
            facb = pool.tile([128, NIMG * 8], F32)
            nc.gpsimd.partition_broadcast(facb[:], fac1[:], channels=128)

            def col(i, k):
                return facb[:, i * 8 + k : i * 8 + k + 1]

            for i in range(NIMG):
                # ---- load ----
                xin = pool.tile([128, F3], F32, tag="io", bufs=2)
                for c in range(3):
                    dma.dma_start(
                        xin[:, c * F : (c + 1) * F],
                        x_h[i, c].flatten().rearrange("(p f) -> p f", p=128),
                    )

                # ---- ph1: brightness + per-channel sums ----
                rgb = pool.tile([128, F3], F16, tag="rgb", bufs=2)
                sums = pool.tile([128, 4], F32, tag="sums", bufs=2)
                jt = pool.tile([128, F3], F16, tag="jt")
                for c in range(3):
                    nc.vector.tensor_scalar(
                        rgb[:, c * F : (c + 1) * F],
                        xin[:, c * F : (c + 1) * F],
                        col(i, 0), 1.0, OP.mult, OP.min,
                    )
                    # per-channel sums for the contrast mean (ACT accum; DVE
                    # tensor_scalar accum_out is broken on HW)
                    nc.scalar.activation(
                        jt[:, c * F : (c + 1) * F],
                        rgb[:, c * F : (c + 1) * F],
                        AF.Identity, bias=0.0, scale=1.0,
                        accum_out=sums[:, c : c + 1],
                    )
                # weighted per-partition sum -> all-core scalar -> delta
                ws = pool.tile([128, 1], F32, tag="ws", bufs=2)
                nc.vector.tensor_scalar(ws[:], sums[:, 0:1], GRAY_W[0], None, OP.mult)
                ws2 = pool.tile([128, 1], F32, tag="ws2", bufs=2)
                nc.vector.scalar_tensor_tensor(ws2[:], sums[:, 1:2], GRAY_W[1], ws[:], OP.mult, OP.add)
                ws3 = pool.tile([128, 1], F32, tag="ws3", bufs=2)
                nc.vector.scalar_tensor_tensor(ws3[:], sums[:, 2:3], GRAY_W[2], ws2[:], OP.mult, OP.add)
                ssum = pool.tile([128, 1], F32, tag="ssum", bufs=2)
                nc.gpsimd.partition_all_reduce(ssum[:], ws3[:], 128, bass_isa.ReduceOp.add)
                delta = pool.tile([128, 1], F32, tag="delta", bufs=2)
                nc.vector.tensor_tensor(delta[:], ssum[:], col(i, 2), OP.mult)

                # ---- ph2: contrast (upper clip fused into ph3 consumers) ----
                ya = pool.tile([128, F3], F16, tag="ya", bufs=2)
                nc.scalar.activation(rgb[:], rgb[:], AF.Relu, bias=delta[:], scale=col(i, 1))

                # ---- ph3: saturation;  gs = (1-sf)*gray(x2) built in ya ----
                # each consumer applies the pending "min 1" via fused 2-scalar TS
                nc.vector.tensor_scalar(ya[:, 0:F], rgb[:, 0:F], 1.0, col(i, 4), OP.min, OP.mult)
                nc.vector.tensor_scalar(ya[:, F:2 * F], rgb[:, F:2 * F], 1.0, col(i, 5), OP.min, OP.mult)
                nc.vector.tensor_tensor(ya[:, 2 * F:3 * F], ya[:, F:2 * F], ya[:, 0:F], OP.add)
                nc.vector.tensor_scalar(ya[:, 0:F], rgb[:, 2 * F:3 * F], 1.0, col(i, 6), OP.min, OP.mult)
                nc.vector.tensor_tensor(ya[:, F:2 * F], ya[:, 0:F], ya[:, 2 * F:3 * F], OP.add)
                y3 = pool.tile([128, F3], F16, tag="y3")
                nc.vector.tensor_scalar(y3[:], rgb[:], 1.0, col(i, 3), OP.min, OP.mult)
                gsb = ya[:, F:2 * F][:, None, :].broadcast_to([128, 3, F])
                nc.vector.tensor_tensor(
                    jt[:].rearrange("p (c f) -> p c f", c=3),
                    y3[:].rearrange("p (c f) -> p c f", c=3),
                    gsb, OP.add,
                )
                nc.vector.tensor_scalar(rgb[:], jt[:], 0.0, 1.0, OP.max, OP.min)

                # ---- ph4: hue ----
                # ya: [0:F]=maxc  [F:2F]=minc  [2F:3F]=cr   (2F used as scratch first)
                nc.vector.tensor_tensor(ya[:, 2 * F:3 * F], rgb[:, 0:F], rgb[:, F:2 * F], OP.max)
                nc.vector.tensor_tensor(ya[:, 0:F], ya[:, 2 * F:3 * F], rgb[:, 2 * F:3 * F], OP.max)
                nc.vector.tensor_tensor(ya[:, 2 * F:3 * F], rgb[:, 0:F], rgb[:, F:2 * F], OP.min)
                nc.vector.tensor_tensor(ya[:, F:2 * F], ya[:, 2 * F:3 * F], rgb[:, 2 * F:3 * F], OP.min)
                nc.vector.tensor_tensor(ya[:, 2 * F:3 * F], ya[:, 0:F], ya[:, F:2 * F], OP.subtract)
                # masks: y3[0:F]=(r>=maxc) y3[F:2F]=(g>=maxc); d1 -> y3[2F:3F]
                mxb = ya[:, 0:F][:, None, :].broadcast_to([128, 2, F])
                nc.vector.tensor_tensor(
                    y3[:, 0:2 * F].bitcast(mybir.dt.int16).rearrange("p (c f) -> p c f", c=2),
                    rgb[:, 0:2 * F].rearrange("p (c f) -> p c f", c=2),
                    mxb, OP.is_ge,
                )
                nc.vector.tensor_tensor(y3[:, 2 * F:3 * F], rgb[:, F:2 * F], rgb[:, 2 * F:3 * F], OP.subtract)
                # jg = 2cr + (b - r)
                nc.vector.tensor_tensor(jt[:, 0:F], rgb[:, 2 * F:3 * F], rgb[:, 0:F], OP.subtract)
                nc.vector.tensor_scalar(jt[:, 2 * F:3 * F], ya[:, 2 * F:3 * F], 2.0, None, OP.mult)
                nc.vector.tensor_tensor(jt[:, F:2 * F], jt[:, 2 * F:3 * F], jt[:, 0:F], OP.add)
                # J = 4cr + (r - g), then case overrides
                nc.vector.tensor_tensor(jt[:, 0:F], rgb[:, 0:F], rgb[:, F:2 * F], OP.subtract)
                nc.vector.tensor_scalar(jt[:, 2 * F:3 * F], ya[:, 2 * F:3 * F], 4.0, None, OP.mult)
                Jt = pool.tile([128, F], F16, tag="Jt")
                nc.vector.tensor_tensor(Jt[:], jt[:, 2 * F:3 * F], jt[:, 0:F], OP.add)
                nc.vector.copy_predicated(Jt[:], y3[:, F:2 * F].bitcast(mybir.dt.int16), jt[:, F:2 * F])
                nc.vector.copy_predicated(Jt[:], y3[:, 0:F].bitcast(mybir.dt.int16), y3[:, 2 * F:3 * F])
                # invc = exp(-ln(cr+eps)) on ACT (f32)
                lc = pool.tile([128, F], F32, tag="lc")
                nc.scalar.activation(lc[:], ya[:, 2 * F:3 * F], AF.Ln, bias=1e-30)
                nc.scalar.activation(lc[:], lc[:], AF.Exp, scale=-1.0)
                # i6 = J*invc + 6hf
                nc.vector.tensor_tensor(jt[:, F:2 * F], Jt[:], lc[:], OP.mult)
                nc.vector.tensor_scalar(jt[:, 0:F], jt[:, F:2 * F], col(i, 7), None, OP.add)
                # recon: A1=|i6+a|, A2=|i6+a-6| per channel (ACT)
                A1 = pool.tile([128, F3], F16, tag="A1")
                A2 = pool.tile([128, F3], F16, tag="A2")
                for ci, a in enumerate((3.0, 1.0, -1.0)):
                    nc.scalar.activation(A1[:, ci * F:(ci + 1) * F], jt[:, 0:F], AF.Abs, bias=a)
                    nc.scalar.activation(A2[:, ci * F:(ci + 1) * F], jt[:, 0:F], AF.Abs, bias=a - 6.0)
                nc.vector.tensor_tensor(y3[:], A1[:], A2[:], OP.min)
                nc.scalar.activation(A1[:], y3[:], AF.Relu, bias=-1.0)
                crb = ya[:, 2 * F:3 * F][:, None, :].broadcast_to([128, 3, F])
                mnb = ya[:, F:2 * F][:, None, :].broadcast_to([128, 3, F])
                mxb3 = ya[:, 0:F][:, None, :].broadcast_to([128, 3, F])
                v3 = lambda t: t[:].rearrange("p (c f) -> p c f", c=3)
                nc.vector.tensor_tensor(v3(A2), v3(A1), crb, OP.mult)
                nc.vector.tensor_tensor(v3(A1), v3(A2), mnb, OP.add)
                nc.vector.tensor_tensor(v3(A2), v3(A1), mxb3, OP.min)
                o3 = pool.tile([128, F3], F32, tag="io", bufs=2)
                nc.scalar.activation(o3[:], A2[:], AF.Copy)

                # ---- store ----
                for c in range(3):
                    dma.dma_start(
                        y_h[i, c].flatten().rearrange("(p f) -> p f", p=128),
                        o3[:, c * F : (c + 1) * F],
                    )

    nc.finalize()
    return nc


def _get_nc():
    if "nc" not in _CACHE:
        _CACHE["nc"] = _build()
    return _CACHE["nc"]


def kernel(x, brightness_f, contrast_f, saturation_f, hue_f, num_samples=1, **_):
    x = np.ascontiguousarray(np.asarray(x, dtype=np.float32))
    bf = np.asarray(brightness_f, np.float32)
    cf = np.asarray(contrast_f, np.float32)
    sf = np.asarray(saturation_f, np.float32)
    hf = np.asarray(hue_f, np.float32)
    B = x.shape[0]
    fac = np.stack(
        [
            bf, cf, (1.0 - cf) / np.float32(NPIX), sf,
            GRAY_W[0] * (1.0 - sf), GRAY_W[1] * (1.0 - sf), GRAY_W[2] * (1.0 - sf),
            6.0 * hf,
        ],
        axis=1,
    ).astype(np.float32)

    nc = _get_nc()
    in_maps = [
        {"x": x[k * NIMG:(k + 1) * NIMG], "fac": fac[k * NIMG:(k + 1) * NIMG]}
        for k in range(8)
    ]
    res = run_bass_kernel_spmd(nc, in_maps, core_ids=list(range(8)), trace=TRACE)
    if TRACE:
        _CACHE["last"] = res
    out = np.concatenate([res.results[k]["y"] for k in range(8)], axis=0)
    return out.astype(np.float32)

